# revision 37
# baseline (speedup 1.0000x reference)
"""8-core SPMD multi-head attention kernel for Trainium2 (Bass/Tile). v2

Problem: nn.MultiHeadAttention, B=2, S=2048, d_model=1024, 16 heads (dk=64).

Sharding: tensor-parallel over heads - 2 heads per core. Q/K/V projection
weights are column-split per core, out-projection row-split; each core
produces a partial [1024, 4096] output that the host sums.

Design (v1 traced at 408us: ACT did 219us incl. copies, PE stuck at the
HAM cold clock behind phase barriers; this version runs ~273us):
  - ACT (scalar engine) runs only the 128 exp activations [128,1024]
    during attention (scale 1/8 and a shift of -2 folded in; softmax is
    shift-invariant). PSUM->SBUF copies / bias adds run on DVE, plus ACT
    only where exp isn't live. One table set, preloaded by a dummy exp.
  - No phase barriers: one PSUM layout for the whole kernel. pss
    [128,1024]x2bufs (4 banks) + pctx [65,1024] (2) + po [128,512]x2 (2).
    Projections, V-relayout and out-projection all rotate the po tag.
  - Attention: for (b, q-window 1024, head): 16 k-chunks of {2 score MMs
    N=512 (tile_position packs the 64-deep contraction) -> one exp
    [128,1024] -> 2 AV MMs accumulating ctx^T + sums via a ones column}.
    AV emission lags 3 chunks so the first AV's pctx WAR (prev block's
    norm chain) doesn't stall the in-order tensor queue.
  - Batch 1's projections drain as side-work units inside batch 0's
    attention chunk stream; out-projection of each q-window rides in the
    second half of the next window; the final window's out-proj splits
    into two 64-contraction MMs reading the h1 staging tile directly.
  - V is re-laid out token-major via identity matmuls; normalization via
    reciprocal_approx_fast + gpsimd partition_broadcast (sums staged to
    SBUF first: reciprocal_approx_fast misreads PSUM inputs).
fp8 was tried and rejected: quantization error on tensors feeding sums
does not average down (~3.5% rel err > the 2e-2 budget).
"""

import sys

sys.path.insert(0, "/opt/trn_rl_repo")

import numpy as np
import ml_dtypes

import concourse.bass as bass  # noqa: F401
import concourse.mybir as mybir
import concourse.tile as tile
from concourse import bacc
from concourse import bass_utils
from concourse.masks import make_identity

B, S, DM, H, DK = 2, 2048, 1024, 16, 64
TOK = B * S
NCORES = 8
HPC = H // NCORES    # 2 heads per core
CW = HPC * DK        # 128 = per-core qkv width
KC = DM // 128       # 8 contraction chunks
KCH = S // 128       # 16 k-token chunks per batch
F32 = mybir.dt.float32
BF16 = mybir.dt.bfloat16
FP8 = mybir.dt.float8e4
AFT = mybir.ActivationFunctionType
DR = mybir.MatmulPerfMode.DoubleRow
VPAD = 80   # fp8 k-tile stride for vh (16B-aligned; holds DK+1=65 used)

TRACE = False
LAST_EXEC_NS = None
LAST_RES = None

_compiled = None


def _build():
    nc = bacc.Bacc("TRN2", target_bir_lowering=False, debug=False,
                   num_devices=NCORES)

    qT = nc.dram_tensor("qT", [DM, TOK], BF16, kind="ExternalInput").ap()
    kT = nc.dram_tensor("kT", [DM, TOK], BF16, kind="ExternalInput").ap()
    vT = nc.dram_tensor("vT", [DM, TOK], BF16, kind="ExternalInput").ap()
    wq = nc.dram_tensor("wq", [128, KC * CW], BF16, kind="ExternalInput").ap()
    wk = nc.dram_tensor("wk", [128, KC * CW], BF16, kind="ExternalInput").ap()
    wv = nc.dram_tensor("wv", [128, KC * CW], BF16, kind="ExternalInput").ap()
    bq = nc.dram_tensor("bq", [128, 1], F32, kind="ExternalInput").ap()
    bk = nc.dram_tensor("bk", [128, 1], F32, kind="ExternalInput").ap()
    bv = nc.dram_tensor("bv", [128, 1], F32, kind="ExternalInput").ap()
    wo = nc.dram_tensor("wo", [CW, DM], BF16, kind="ExternalInput").ap()
    bo8 = nc.dram_tensor("bo8", [128, 8], F32, kind="ExternalInput").ap()
    out = nc.dram_tensor("out", [DM, TOK], BF16, kind="ExternalOutput").ap()

    xsrc = {"q": qT, "k": kT, "v": vT}

    with tile.TileContext(nc) as tc, \
         tc.tile_pool(name="const", bufs=1) as const, \
         tc.tile_pool(name="xin", bufs=2) as xin, \
         tc.tile_pool(name="expp", bufs=10) as expp, \
         tc.tile_pool(name="stage", bufs=2) as stage, \
         tc.tile_pool(name="outst", bufs=8) as outst, \
         tc.tile_pool(name="ppool", bufs=2, space="PSUM") as ppool, \
         tc.tile_pool(name="pscore", bufs=2, space="PSUM") as pscore, \
         tc.tile_pool(name="pctxp", bufs=1, space="PSUM") as pctxp:

        # ---------- constants & persistent buffers ----------
        # weight DMAs are emitted inside the proj scope so the FIFO DMA
        # ring streams batch-0's K input first (it gates the first MMs)
        w_sb = {}
        b_sb = {}
        wsrc = {"q": wq, "k": wk, "v": wv}
        bsrc = {"q": bq, "k": bk, "v": bv}
        for name in ("q", "k", "v"):
            w_sb[name] = const.tile([128, KC, CW], BF16, tag=f"w{name}",
                                    name=f"w{name}")
            b_sb[name] = const.tile([128, 1], F32, tag=f"b{name}",
                                    name=f"b{name}")

        def weight_dmas(name):
            nc.sync.dma_start(w_sb[name][:],
                              wsrc[name].rearrange("p (c m) -> p c m", c=KC))
            nc.sync.dma_start(b_sb[name][:], bsrc[name][:])

        wo_sb = const.tile([CW, DM], BF16, tag="wo")
        # lower half of Wo re-homed at partitions 0-63: lets the final
        # window's out-proj read the h1 ctx from its staging tile (base 0)
        # via a second 64-contraction accumulate, skipping the ctxT DMA
        wo2_sb = const.tile([DK, DM], BF16, tag="wo2")
        bo_sb = const.tile([128, 8], F32, tag="bo")
        ident = const.tile([128, 128], BF16, tag="ident")
        make_identity(nc, ident[:])

        qhT = [const.tile([128, S], BF16, tag=f"qhT{b}", name=f"qhT{b}")
               for b in range(B)]
        khT = [const.tile([128, S], BF16, tag=f"khT{b}", name=f"khT{b}")
               for b in range(B)]
        vhT = [const.tile([128, S], BF16, tag=f"vhT{b}", name=f"vhT{b}")
               for b in range(B)]
        ctxT = [const.tile([128, S], BF16, tag=f"ctxT{b}", name=f"ctxT{b}")
                for b in range(B)]
        dstT = {"q": qhT, "k": khT, "v": vhT}
        # vh: token-major V with a ones column for the softmax sums
        vh = const.tile([128, B, HPC, KCH, DK + 1], BF16, tag="vh")
        nc.gpsimd.memset(vh[:, :, :, :, DK:DK + 1], 1.0)

        # per-partition -2.0 bias column for the shifted exp
        negtwo = const.tile([128, 1], F32, tag="negtwo")
        nc.gpsimd.memset(negtwo[:], -2.0)

        # dummy exp: trigger the one ACT_TABLE_LOAD during the prologue
        warm = const.tile([1, 1], F32, tag="warm")
        nc.scalar.activation(warm[:], negtwo[0:1, 0:1], AFT.Exp, scale=0.125)

        # ---------- phase 1: projections (k,q then v per batch) ----------
        # po-tag PSUM tiles [128,512] rotate through proj, V-transpose and
        # out-projection uses; alternate DVE/GpSimd for the bias copies.
        copy_flip = 0

        def input_dmas(b, name):
            xt = xin.tile([128, KC, S], BF16, tag="xt", name=f"xt_{name}{b}")
            for c in range(KC):
                nc.sync.dma_start(
                    xt[:, c, :],
                    xsrc[name][c * 128:(c + 1) * 128, b * S:(b + 1) * S])
            return xt

        def proj_tile(b, name, xt, qtr, act_ok):
            nonlocal copy_flip
            pp = ppool.tile([128, 512], F32, tag="po",
                            name=f"pp_{name}{b}_{qtr}")
            for c in range(KC):
                nc.tensor.matmul(
                    pp[:], w_sb[name][:, c, :],
                    xt[:, c, qtr * 512:(qtr + 1) * 512],
                    start=(c == 0), stop=(c == KC - 1))
            d = dstT[name][b][:, qtr * 512:(qtr + 1) * 512]
            if act_ok and copy_flip % 2 == 1:
                nc.scalar.activation(d, pp[:], AFT.Identity,
                                     bias=b_sb[name][:])
            else:
                nc.vector.tensor_scalar_add(d, pp[:], b_sb[name][:])
            copy_flip += 1

        def relayout_tile(b, t, act_ok):
            # vh[tok, dims] = vhT^T via identity matmul, then copy
            nonlocal copy_flip
            pt = ppool.tile([128, 512], F32, tag="po", name=f"pt{b}_{t}")
            nc.tensor.matmul(pt[:, 0:128],
                             vhT[b][:, t * 128:(t + 1) * 128], ident[:])
            src = pt[:, 0:128].rearrange("p (h d) -> p h d", h=HPC)
            dst = vh[:, b, :, t, 0:DK]
            if act_ok and copy_flip % 2 == 1:
                nc.scalar.activation(dst, src, AFT.Copy)
            else:
                nc.vector.tensor_copy(dst, src)
            copy_flip += 1

        # prologue: batch 0 projections serial (ACT is free to help);
        # batch 1 projections become side-work units drained one per
        # chunk inside batch 0's attention stream.
        with nc.named_scope("proj"):
            xts = {}
            weight_dmas("k")
            xts["k"] = input_dmas(0, "k")
            weight_dmas("v")
            weight_dmas("q")
            for nm in ("v", "q"):
                xts[nm] = input_dmas(0, nm)
            # out-proj weights: first needed ~90us in, after b0's inputs
            nc.sync.dma_start(wo_sb[:], wo[:])
            nc.sync.dma_start(wo2_sb[:], wo[DK:2 * DK, :])
            nc.sync.dma_start(bo_sb[:], bo8[:])
            for qtr in range(4):
                proj_tile(0, "k", xts["k"], qtr, act_ok=True)
            for qtr in range(4):
                proj_tile(0, "v", xts["v"], qtr, act_ok=True)
            for t in range(KCH):
                relayout_tile(0, t, act_ok=True)
            for qtr in range(4):
                proj_tile(0, "q", xts["q"], qtr, act_ok=True)
            for nm in ("k", "v", "q"):
                xts[nm] = input_dmas(1, nm)

        side_work = []
        for qtr in range(4):
            side_work.append(lambda qtr=qtr: proj_tile(
                1, "k", xts["k"], qtr, act_ok=False))
        for vt in range(4):
            side_work.append(lambda vt=vt: proj_tile(
                1, "v", xts["v"], vt, act_ok=False))
            for t in range(vt * 4, vt * 4 + 4):
                side_work.append(lambda t=t: relayout_tile(
                    1, t, act_ok=False))
        for qtr in range(4):
            side_work.append(lambda qtr=qtr: proj_tile(
                1, "q", xts["q"], qtr, act_ok=False))
        side_work.reverse()   # pop() from the front-most unit

        # ---------- phase 2: attention with interleaved out-proj ----------
        # blocks: (b, qw, h); 16 chunks each. Out-proj of the previous
        # q-window (8 od-matmuls) rides along every 4th chunk.
        oproj_queue = []   # (b, qw) blocks whose ctxT is complete
        cs_tiles = {}      # (b, qw) -> h1 ctx staging tile

        def emit_oproj(n, act_ok=False):
            # emit n pending out-projection column-chunks, if any
            for i in range(n):
                if not oproj_queue:
                    return
                ob, oqw, od = oproj_queue.pop(0)
                col = od // 2
                po = ppool.tile([128, 512], F32, tag="po",
                                name=f"po{ob}_{oqw}_{od}")
                q0 = oqw * 1024 + (od % 2) * 512
                if act_ok and (ob, oqw) in cs_tiles:
                    cs = cs_tiles[(ob, oqw)]
                    nc.tensor.matmul(
                        po[:], wo_sb[0:DK, col * 128:(col + 1) * 128],
                        ctxT[ob][0:DK, q0:q0 + 512],
                        start=True, stop=False)
                    nc.tensor.matmul(
                        po[:], wo2_sb[:, col * 128:(col + 1) * 128],
                        cs[:, (od % 2) * 512:(od % 2) * 512 + 512],
                        start=False, stop=True)
                else:
                    nc.tensor.matmul(
                        po[:], wo_sb[:, col * 128:(col + 1) * 128],
                        ctxT[ob][:, q0:q0 + 512])
                ot = outst.tile([128, 512], BF16, tag="ot",
                                name=f"ot{ob}_{oqw}_{od}")
                if act_ok and i % 2 == 1:
                    nc.scalar.activation(ot[:], po[:], AFT.Identity,
                                         bias=bo_sb[:, col:col + 1])
                else:
                    nc.vector.tensor_scalar_add(
                        ot[:], po[:], bo_sb[:, col:col + 1])
                odma = nc.gpsimd if (act_ok and i % 2 == 0) else nc.sync
                odma.dma_start(
                    out[col * 128:(col + 1) * 128,
                        ob * S + q0:ob * S + q0 + 512], ot[:])

        with nc.named_scope("attn"):
            for b in range(B):
                for qw in range(2):
                    q0 = qw * 1024
                    for h in range(HPC):
                        hb = h * DK
                        pctx = pctxp.tile([DK + 1, 1024], F32, tag="pctx",
                                          name=f"pctx{b}_{qw}_{h}")
                        etiles = {}

                        def do_av(c):
                            ec = etiles.pop(c)
                            for qn in range(2):
                                nc.tensor.matmul(
                                    pctx[:, qn * 512:(qn + 1) * 512],
                                    vh[:, b, h, c, :],
                                    ec[:, qn * 512:(qn + 1) * 512],
                                    start=(c == 0), stop=(c == KCH - 1))

                        # AV emission lags the score/exp stream by AVLAG
                        # chunks so the first AV (which WAR-waits the prev
                        # block's pctx through its norm chain) enters the
                        # in-order tensor queue after that chain has drained.
                        AVLAG = 3
                        for c in range(KCH):
                            pss = pscore.tile([128, 1024], F32, tag="pss",
                                              name=f"pss{b}_{qw}_{h}_{c}")
                            for qn in range(2):
                                nc.tensor.matmul(
                                    pss[:, qn * 512:(qn + 1) * 512],
                                    khT[b][hb:hb + DK,
                                           c * 128:(c + 1) * 128],
                                    qhT[b][hb:hb + DK,
                                           q0 + qn * 512:q0 + (qn + 1) * 512],
                                    tile_position=(hb, 0))
                            e = expp.tile([128, 1024], BF16, tag="e",
                                          name=f"e{b}_{qw}_{h}_{c}")
                            # bias -2: softmax is shift-invariant (the sums
                            # row scales identically); keeps exp small
                            nc.scalar.activation(e[:], pss[:], AFT.Exp,
                                                 bias=negtwo[:], scale=0.125)
                            etiles[c] = e
                            if c >= AVLAG:
                                do_av(c - AVLAG)
                            # batch-1 projection side-work keeps the PE fed
                            # through batch 0's attention; slots track the
                            # input-DMA arrival order (in-order PE queue).
                            gchunk = (qw * 2 + h) * KCH + c
                            if b == 0 and 8 <= gchunk and side_work:
                                side_work.pop()()
                            # out-proj of the previous q-window rides in the
                            # second half of this window so its ctxT (incl.
                            # the norm chain + h1 DMA) is ready — an early
                            # emit stalls the in-order tensor queue.
                            if 8 <= (h * KCH + c) < 24:
                                emit_oproj(1)
                        for c in range(KCH - AVLAG, KCH):
                            do_av(c)
                        # normalization: ctx / sums (row DK of pctx).
                        # NB: reciprocal_approx_fast reading PSUM directly
                        # gives garbage — stage the sums row in SBUF first.
                        # Split per q-half so the first consumer unblocks
                        # after half the chain latency.
                        ssum = stage.tile([1, 1024], F32, tag=f"ssum{h}",
                                          name=f"ssum{b}_{qw}_{h}")
                        si = stage.tile([1, 1024], F32, tag=f"si{h}",
                                        name=f"si{b}_{qw}_{h}")
                        sbc = stage.tile([DK, 1024], F32, tag=f"sbc{h}",
                                         name=f"sbc{b}_{qw}_{h}")
                        if h == 1:
                            cs = stage.tile([DK, 1024], BF16, tag="cs",
                                            name=f"cs{b}_{qw}")
                        for qn in range(2):
                            sl = slice(qn * 512, (qn + 1) * 512)
                            nc.vector.tensor_copy(ssum[:, sl],
                                                  pctx[DK:DK + 1, sl])
                            nc.vector.reciprocal_approx_fast(si[:, sl],
                                                             ssum[:, sl])
                            nc.gpsimd.partition_broadcast(sbc[:, sl],
                                                          si[:, sl])
                            if h == 0:
                                nc.vector.tensor_mul(
                                    ctxT[b][0:DK,
                                            q0 + qn * 512:q0 + qn * 512
                                            + 512],
                                    pctx[0:DK, sl], sbc[:, sl])
                            else:
                                nc.vector.tensor_mul(cs[:, sl],
                                                     pctx[0:DK, sl],
                                                     sbc[:, sl])
                        if h == 1:
                            if (b, qw) == (B - 1, 1):
                                # final window: tail out-proj reads cs
                                # directly; no DMA needed
                                cs_tiles[(b, qw)] = cs
                            else:
                                # gpsimd SWDGE ring: keeps this off the
                                # sync ring so the next window's out-proj
                                # isn't stuck behind queued output DMAs
                                nc.gpsimd.dma_start(
                                    ctxT[b][DK:2 * DK, q0:q0 + 1024], cs[:])
                    for od in range(16):
                        oproj_queue.append((b, qw, od))

        with nc.named_scope("tail"):
            emit_oproj(len(oproj_queue), act_ok=True)

    nc.compile()
    return nc


def _get_compiled():
    global _compiled
    if _compiled is None:
        _compiled = _build()
    return _compiled


def _xT(x):
    xf = np.asarray(x, np.float32).reshape(TOK, DM)
    return np.ascontiguousarray(xf.T).astype(ml_dtypes.bfloat16)


def _wshuf(W, cs):
    # [1024, 128] core slice -> [p, c*128+m] so SBUF [128, KC, CW] DMAs clean
    Wc = np.asarray(W, np.float32)[:, cs]
    return np.ascontiguousarray(
        Wc.reshape(KC, 128, CW).transpose(1, 0, 2).reshape(128, KC * CW)
    ).astype(ml_dtypes.bfloat16)


def kernel(q, k, v, Wq, bq, Wk, bk, Wv, bv, Wo, bo):
    global LAST_EXEC_NS, LAST_RES
    nc = _get_compiled()

    qTa, kTa, vTa = _xT(q), _xT(k), _xT(v)

    bq, bk, bv = (np.asarray(a, np.float32) for a in (bq, bk, bv))
    Wo = np.asarray(Wo, np.float32)
    bo = np.asarray(bo, np.float32)

    in_maps = []
    for c in range(NCORES):
        cs = slice(c * CW, (c + 1) * CW)
        in_maps.append({
            "qT": qTa, "kT": kTa, "vT": vTa,
            "wq": _wshuf(Wq, cs), "wk": _wshuf(Wk, cs), "wv": _wshuf(Wv, cs),
            "bq": np.ascontiguousarray(bq[cs].reshape(CW, 1)),
            "bk": np.ascontiguousarray(bk[cs].reshape(CW, 1)),
            "bv": np.ascontiguousarray(bv[cs].reshape(CW, 1)),
            "wo": np.ascontiguousarray(Wo[cs, :]).astype(ml_dtypes.bfloat16),
            "bo8": np.ascontiguousarray((bo / NCORES).reshape(8, 128).T),
        })

    kwargs = {}
    if TRACE:
        try:
            import os
            import shutil
            import ntff_shim
            ntff_shim.install()
            kwargs["trace"] = True
            tdir = "/tmp/bass_trace"
            shutil.rmtree(tdir, ignore_errors=True)
            os.makedirs(tdir, exist_ok=True)
            kwargs["tmpdir"] = tdir
        except Exception as e:
            print(f"trace setup failed: {e}")

    res = bass_utils.run_bass_kernel_spmd(
        nc, in_maps, core_ids=list(range(NCORES)), **kwargs)
    LAST_EXEC_NS = res.exec_time_ns
    LAST_RES = res

    total = res.results[0]["out"].astype(np.float32).copy()
    for c in range(1, NCORES):
        total += res.results[c]["out"]
    return np.ascontiguousarray(total.T).reshape(B, S, DM)


# revision 38
# speedup vs baseline: 1.0814x; 1.0814x over previous
"""8-core SPMD multi-head attention kernel for Trainium2 (Bass/Tile). v2

Problem: nn.MultiHeadAttention, B=2, S=2048, d_model=1024, 16 heads (dk=64).

Sharding: tensor-parallel over heads - 2 heads per core. Q/K/V projection
weights are column-split per core, out-projection row-split; each core
produces a partial [1024, 4096] output that the host sums.

Design (v1 traced at 408us: ACT did 219us incl. copies, PE stuck at the
HAM cold clock behind phase barriers; this version runs ~273us):
  - ACT (scalar engine) runs only the 128 exp activations [128,1024]
    during attention (scale 1/8 and a shift of -2 folded in; softmax is
    shift-invariant). PSUM->SBUF copies / bias adds run on DVE, plus ACT
    only where exp isn't live. One table set, preloaded by a dummy exp.
  - No phase barriers: one PSUM layout for the whole kernel. pss
    [128,1024]x2bufs (4 banks) + pctx [65,1024] (2) + po [128,512]x2 (2).
    Projections, V-relayout and out-projection all rotate the po tag.
  - Attention: for (b, q-window 1024, head): 16 k-chunks of {2 score MMs
    N=512 (tile_position packs the 64-deep contraction) -> one exp
    [128,1024] -> 2 AV MMs accumulating ctx^T + sums via a ones column}.
    AV emission lags 3 chunks so the first AV's pctx WAR (prev block's
    norm chain) doesn't stall the in-order tensor queue.
  - Batch 1's projections drain as side-work units inside batch 0's
    attention chunk stream; out-projection of each q-window rides in the
    second half of the next window; the final window's out-proj splits
    into two 64-contraction MMs reading the h1 staging tile directly.
  - V is re-laid out token-major via identity matmuls; normalization via
    reciprocal_approx_fast + gpsimd partition_broadcast (sums staged to
    SBUF first: reciprocal_approx_fast misreads PSUM inputs).
fp8 was tried and rejected: quantization error on tensors feeding sums
does not average down (~3.5% rel err > the 2e-2 budget).
"""

import sys

sys.path.insert(0, "/opt/trn_rl_repo")

import numpy as np
import ml_dtypes

import concourse.bass as bass  # noqa: F401
import concourse.mybir as mybir
import concourse.tile as tile
from concourse import bacc
from concourse import bass_utils
from concourse.masks import make_identity

B, S, DM, H, DK = 2, 2048, 1024, 16, 64
TOK = B * S
NCORES = 8
HPC = H // NCORES    # 2 heads per core
CW = HPC * DK        # 128 = per-core qkv width
KC = DM // 128       # 8 contraction chunks
KCH = S // 128       # 16 k-token chunks per batch
F32 = mybir.dt.float32
BF16 = mybir.dt.bfloat16
FP8 = mybir.dt.float8e4
AFT = mybir.ActivationFunctionType
DR = mybir.MatmulPerfMode.DoubleRow
VPAD = 80   # fp8 k-tile stride for vh (16B-aligned; holds DK+1=65 used)

TRACE = False
LAST_EXEC_NS = None
LAST_RES = None

_compiled = None


def _build():
    nc = bacc.Bacc("TRN2", target_bir_lowering=False, debug=False,
                   num_devices=NCORES)

    qT = nc.dram_tensor("qT", [DM, TOK], BF16, kind="ExternalInput").ap()
    kT = nc.dram_tensor("kT", [DM, TOK], BF16, kind="ExternalInput").ap()
    vT = nc.dram_tensor("vT", [DM, TOK], BF16, kind="ExternalInput").ap()
    wq = nc.dram_tensor("wq", [128, KC * CW], BF16, kind="ExternalInput").ap()
    wk = nc.dram_tensor("wk", [128, KC * CW], BF16, kind="ExternalInput").ap()
    wv = nc.dram_tensor("wv", [128, KC * CW], BF16, kind="ExternalInput").ap()
    bq = nc.dram_tensor("bq", [128, 1], F32, kind="ExternalInput").ap()
    bk = nc.dram_tensor("bk", [128, 1], F32, kind="ExternalInput").ap()
    bv = nc.dram_tensor("bv", [128, 1], F32, kind="ExternalInput").ap()
    wo = nc.dram_tensor("wo", [CW, DM], BF16, kind="ExternalInput").ap()
    bo8 = nc.dram_tensor("bo8", [128, 8], F32, kind="ExternalInput").ap()
    out = nc.dram_tensor("out", [DM, TOK], BF16, kind="ExternalOutput").ap()

    xsrc = {"q": qT, "k": kT, "v": vT}

    with tile.TileContext(nc) as tc, \
         tc.tile_pool(name="const", bufs=1) as const, \
         tc.tile_pool(name="xin", bufs=2) as xin, \
         tc.tile_pool(name="expp", bufs=10) as expp, \
         tc.tile_pool(name="stage", bufs=2) as stage, \
         tc.tile_pool(name="outst", bufs=8) as outst, \
         tc.tile_pool(name="ppool", bufs=2, space="PSUM") as ppool, \
         tc.tile_pool(name="pscore", bufs=2, space="PSUM") as pscore, \
         tc.tile_pool(name="pctxp", bufs=1, space="PSUM") as pctxp:

        # ---------- constants & persistent buffers ----------
        # weight DMAs are emitted inside the proj scope so the FIFO DMA
        # ring streams batch-0's K input first (it gates the first MMs)
        w_sb = {}
        b_sb = {}
        wsrc = {"q": wq, "k": wk, "v": wv}
        bsrc = {"q": bq, "k": bk, "v": bv}
        for name in ("q", "k", "v"):
            w_sb[name] = const.tile([128, KC, CW], BF16, tag=f"w{name}",
                                    name=f"w{name}")
            b_sb[name] = const.tile([128, 1], F32, tag=f"b{name}",
                                    name=f"b{name}")

        def weight_dmas(name):
            nc.sync.dma_start(w_sb[name][:],
                              wsrc[name].rearrange("p (c m) -> p c m", c=KC))
            nc.sync.dma_start(b_sb[name][:], bsrc[name][:])

        wo_sb = const.tile([CW, DM], BF16, tag="wo")
        # lower half of Wo re-homed at partitions 0-63: lets the final
        # window's out-proj read the h1 ctx from its staging tile (base 0)
        # via a second 64-contraction accumulate, skipping the ctxT DMA
        wo2_sb = const.tile([DK, DM], BF16, tag="wo2")
        bo_sb = const.tile([128, 8], F32, tag="bo")
        ident = const.tile([128, 128], BF16, tag="ident")
        make_identity(nc, ident[:])

        qhT = [const.tile([128, S], BF16, tag=f"qhT{b}", name=f"qhT{b}")
               for b in range(B)]
        khT = [const.tile([128, S], BF16, tag=f"khT{b}", name=f"khT{b}")
               for b in range(B)]
        vhT = [const.tile([128, S], BF16, tag=f"vhT{b}", name=f"vhT{b}")
               for b in range(B)]
        ctxT = [const.tile([128, S], BF16, tag=f"ctxT{b}", name=f"ctxT{b}")
                for b in range(B)]
        dstT = {"q": qhT, "k": khT, "v": vhT}
        # vh: token-major V with a ones column for the softmax sums
        vh = const.tile([128, B, HPC, KCH, DK + 1], BF16, tag="vh")
        nc.gpsimd.memset(vh[:, :, :, :, DK:DK + 1], 1.0)

        # per-partition -2.0 bias column for the shifted exp
        negtwo = const.tile([128, 1], F32, tag="negtwo")
        nc.gpsimd.memset(negtwo[:], -2.0)

        # dummy exp: trigger the one ACT_TABLE_LOAD during the prologue
        warm = const.tile([1, 1], F32, tag="warm")
        nc.scalar.activation(warm[:], negtwo[0:1, 0:1], AFT.Exp, scale=0.125)

        # ---------- phase 1: projections (k,q then v per batch) ----------
        # po-tag PSUM tiles [128,512] rotate through proj, V-transpose and
        # out-projection uses; alternate DVE/GpSimd for the bias copies.
        copy_flip = 0

        def input_dmas(b, name):
            xt = xin.tile([128, KC, S], BF16, tag="xt", name=f"xt_{name}{b}")
            for c in range(KC):
                nc.sync.dma_start(
                    xt[:, c, :],
                    xsrc[name][c * 128:(c + 1) * 128, b * S:(b + 1) * S])
            return xt

        def proj_tile(b, name, xt, qtr, act_ok):
            nonlocal copy_flip
            pp = ppool.tile([128, 512], F32, tag="po",
                            name=f"pp_{name}{b}_{qtr}")
            for c in range(KC):
                nc.tensor.matmul(
                    pp[:], w_sb[name][:, c, :],
                    xt[:, c, qtr * 512:(qtr + 1) * 512],
                    start=(c == 0), stop=(c == KC - 1))
            d = dstT[name][b][:, qtr * 512:(qtr + 1) * 512]
            if act_ok and copy_flip % 2 == 1:
                nc.scalar.activation(d, pp[:], AFT.Identity,
                                     bias=b_sb[name][:])
            else:
                nc.vector.tensor_scalar_add(d, pp[:], b_sb[name][:])
            copy_flip += 1

        def relayout_tile(b, t, act_ok):
            # vh[tok, dims] = vhT^T via identity matmul, then copy
            nonlocal copy_flip
            pt = ppool.tile([128, 512], F32, tag="po", name=f"pt{b}_{t}")
            nc.tensor.matmul(pt[:, 0:128],
                             vhT[b][:, t * 128:(t + 1) * 128], ident[:])
            src = pt[:, 0:128].rearrange("p (h d) -> p h d", h=HPC)
            dst = vh[:, b, :, t, 0:DK]
            if act_ok and copy_flip % 2 == 1:
                nc.scalar.activation(dst, src, AFT.Copy)
            else:
                nc.vector.tensor_copy(dst, src)
            copy_flip += 1

        # prologue: batch 0 projections serial (ACT is free to help);
        # batch 1 projections become side-work units drained one per
        # chunk inside batch 0's attention stream.
        with nc.named_scope("proj"):
            xts = {}
            weight_dmas("k")
            xts["k"] = input_dmas(0, "k")
            weight_dmas("v")
            weight_dmas("q")
            for nm in ("v", "q"):
                xts[nm] = input_dmas(0, nm)
            # out-proj weights: first needed ~90us in, after b0's inputs
            nc.sync.dma_start(wo_sb[:], wo[:])
            nc.sync.dma_start(wo2_sb[:], wo[DK:2 * DK, :])
            nc.sync.dma_start(bo_sb[:], bo8[:])
            for qtr in range(4):
                proj_tile(0, "k", xts["k"], qtr, act_ok=True)
            for qtr in range(4):
                proj_tile(0, "v", xts["v"], qtr, act_ok=True)
            for t in range(KCH):
                relayout_tile(0, t, act_ok=True)
            for qtr in range(4):
                proj_tile(0, "q", xts["q"], qtr, act_ok=True)
            for nm in ("k", "v", "q"):
                xts[nm] = input_dmas(1, nm)

        side_work = []
        for qtr in range(4):
            side_work.append(lambda qtr=qtr: proj_tile(
                1, "k", xts["k"], qtr, act_ok=False))
        for vt in range(4):
            side_work.append(lambda vt=vt: proj_tile(
                1, "v", xts["v"], vt, act_ok=False))
            for t in range(vt * 4, vt * 4 + 4):
                side_work.append(lambda t=t: relayout_tile(
                    1, t, act_ok=False))
        for qtr in range(4):
            side_work.append(lambda qtr=qtr: proj_tile(
                1, "q", xts["q"], qtr, act_ok=False))
        side_work.reverse()   # pop() from the front-most unit

        # ---------- phase 2: attention with interleaved out-proj ----------
        # blocks: (b, qw, h); 16 chunks each. Out-proj of the previous
        # q-window (8 od-matmuls) rides along every 4th chunk.
        oproj_queue = []   # (b, qw) blocks whose ctxT is complete
        cs_tiles = {}      # (b, qw) -> h1 ctx staging tile

        def emit_oproj(n, act_ok=False):
            # emit n pending out-projection column-chunks, if any
            for i in range(n):
                if not oproj_queue:
                    return
                ob, oqw, od = oproj_queue.pop(0)
                col = od // 2
                po = ppool.tile([128, 512], F32, tag="po",
                                name=f"po{ob}_{oqw}_{od}")
                q0 = oqw * 1024 + (od % 2) * 512
                if act_ok and (ob, oqw) in cs_tiles:
                    cs = cs_tiles[(ob, oqw)]
                    nc.tensor.matmul(
                        po[:], wo_sb[0:DK, col * 128:(col + 1) * 128],
                        ctxT[ob][0:DK, q0:q0 + 512],
                        start=True, stop=False)
                    nc.tensor.matmul(
                        po[:], wo2_sb[:, col * 128:(col + 1) * 128],
                        cs[:, (od % 2) * 512:(od % 2) * 512 + 512],
                        start=False, stop=True)
                else:
                    nc.tensor.matmul(
                        po[:], wo_sb[:, col * 128:(col + 1) * 128],
                        ctxT[ob][:, q0:q0 + 512])
                ot = outst.tile([128, 512], BF16, tag="ot",
                                name=f"ot{ob}_{oqw}_{od}")
                if act_ok and i % 2 == 1:
                    nc.scalar.activation(ot[:], po[:], AFT.Identity,
                                         bias=bo_sb[:, col:col + 1])
                else:
                    nc.vector.tensor_scalar_add(
                        ot[:], po[:], bo_sb[:, col:col + 1])
                nc.sync.dma_start(
                    out[col * 128:(col + 1) * 128,
                        ob * S + q0:ob * S + q0 + 512], ot[:])

        with nc.named_scope("attn"):
            for b in range(B):
                for qw in range(2):
                    q0 = qw * 1024
                    for h in range(HPC):
                        hb = h * DK
                        pctx = pctxp.tile([DK + 1, 1024], F32, tag="pctx",
                                          name=f"pctx{b}_{qw}_{h}")
                        etiles = {}

                        def do_av(c):
                            ec = etiles.pop(c)
                            for qn in range(2):
                                nc.tensor.matmul(
                                    pctx[:, qn * 512:(qn + 1) * 512],
                                    vh[:, b, h, c, :],
                                    ec[:, qn * 512:(qn + 1) * 512],
                                    start=(c == 0), stop=(c == KCH - 1))

                        # AV emission lags the score/exp stream by AVLAG
                        # chunks so the first AV (which WAR-waits the prev
                        # block's pctx through its norm chain) enters the
                        # in-order tensor queue after that chain has drained.
                        AVLAG = 3
                        for c in range(KCH):
                            pss = pscore.tile([128, 1024], F32, tag="pss",
                                              name=f"pss{b}_{qw}_{h}_{c}")
                            for qn in range(2):
                                nc.tensor.matmul(
                                    pss[:, qn * 512:(qn + 1) * 512],
                                    khT[b][hb:hb + DK,
                                           c * 128:(c + 1) * 128],
                                    qhT[b][hb:hb + DK,
                                           q0 + qn * 512:q0 + (qn + 1) * 512],
                                    tile_position=(hb, 0))
                            e = expp.tile([128, 1024], BF16, tag="e",
                                          name=f"e{b}_{qw}_{h}_{c}")
                            # bias -2: softmax is shift-invariant (the sums
                            # row scales identically); keeps exp small
                            nc.scalar.activation(e[:], pss[:], AFT.Exp,
                                                 bias=negtwo[:], scale=0.125)
                            etiles[c] = e
                            if c >= AVLAG:
                                do_av(c - AVLAG)
                            # batch-1 projection side-work keeps the PE fed
                            # through batch 0's attention; slots track the
                            # input-DMA arrival order (in-order PE queue).
                            gchunk = (qw * 2 + h) * KCH + c
                            if b == 0 and 8 <= gchunk and side_work:
                                side_work.pop()()
                            # out-proj of the previous q-window rides in the
                            # second half of this window so its ctxT (incl.
                            # the norm chain + h1 DMA) is ready — an early
                            # emit stalls the in-order tensor queue.
                            if 8 <= (h * KCH + c) < 24:
                                emit_oproj(1)
                        for c in range(KCH - AVLAG, KCH):
                            do_av(c)
                        # normalization: ctx / sums (row DK of pctx).
                        # NB: reciprocal_approx_fast reading PSUM directly
                        # gives garbage — stage the sums row in SBUF first.
                        ssum = stage.tile([1, 1024], F32, tag=f"ssum{h}",
                                          name=f"ssum{b}_{qw}_{h}")
                        nc.vector.tensor_copy(ssum[:], pctx[DK:DK + 1, :])
                        si = stage.tile([1, 1024], F32, tag=f"si{h}",
                                        name=f"si{b}_{qw}_{h}")
                        nc.vector.reciprocal_approx_fast(si[:], ssum[:])
                        sbc = stage.tile([DK, 1024], F32, tag=f"sbc{h}",
                                         name=f"sbc{b}_{qw}_{h}")
                        nc.gpsimd.partition_broadcast(sbc[:], si[:])
                        if h == 0:
                            nc.vector.tensor_mul(
                                ctxT[b][0:DK, q0:q0 + 1024],
                                pctx[0:DK, :], sbc[:])
                        else:
                            cs = stage.tile([DK, 1024], BF16, tag="cs",
                                            name=f"cs{b}_{qw}")
                            nc.vector.tensor_mul(cs[:], pctx[0:DK, :],
                                                 sbc[:])
                        if h == 1:
                            if (b, qw) == (B - 1, 1):
                                # final window: tail out-proj reads cs
                                # directly; no DMA needed
                                cs_tiles[(b, qw)] = cs
                            else:
                                # gpsimd SWDGE ring: keeps this off the
                                # sync ring so the next window's out-proj
                                # isn't stuck behind queued output DMAs
                                nc.gpsimd.dma_start(
                                    ctxT[b][DK:2 * DK, q0:q0 + 1024], cs[:])
                    for od in range(16):
                        oproj_queue.append((b, qw, od))

        with nc.named_scope("tail"):
            emit_oproj(len(oproj_queue), act_ok=True)

    nc.compile()
    return nc


def _get_compiled():
    global _compiled
    if _compiled is None:
        _compiled = _build()
    return _compiled


def _xT(x):
    xf = np.asarray(x, np.float32).reshape(TOK, DM)
    return np.ascontiguousarray(xf.T).astype(ml_dtypes.bfloat16)


def _wshuf(W, cs):
    # [1024, 128] core slice -> [p, c*128+m] so SBUF [128, KC, CW] DMAs clean
    Wc = np.asarray(W, np.float32)[:, cs]
    return np.ascontiguousarray(
        Wc.reshape(KC, 128, CW).transpose(1, 0, 2).reshape(128, KC * CW)
    ).astype(ml_dtypes.bfloat16)


def kernel(q, k, v, Wq, bq, Wk, bk, Wv, bv, Wo, bo):
    global LAST_EXEC_NS, LAST_RES
    nc = _get_compiled()

    qTa, kTa, vTa = _xT(q), _xT(k), _xT(v)

    bq, bk, bv = (np.asarray(a, np.float32) for a in (bq, bk, bv))
    Wo = np.asarray(Wo, np.float32)
    bo = np.asarray(bo, np.float32)

    in_maps = []
    for c in range(NCORES):
        cs = slice(c * CW, (c + 1) * CW)
        in_maps.append({
            "qT": qTa, "kT": kTa, "vT": vTa,
            "wq": _wshuf(Wq, cs), "wk": _wshuf(Wk, cs), "wv": _wshuf(Wv, cs),
            "bq": np.ascontiguousarray(bq[cs].reshape(CW, 1)),
            "bk": np.ascontiguousarray(bk[cs].reshape(CW, 1)),
            "bv": np.ascontiguousarray(bv[cs].reshape(CW, 1)),
            "wo": np.ascontiguousarray(Wo[cs, :]).astype(ml_dtypes.bfloat16),
            "bo8": np.ascontiguousarray((bo / NCORES).reshape(8, 128).T),
        })

    kwargs = {}
    if TRACE:
        try:
            import os
            import shutil
            import ntff_shim
            ntff_shim.install()
            kwargs["trace"] = True
            tdir = "/tmp/bass_trace"
            shutil.rmtree(tdir, ignore_errors=True)
            os.makedirs(tdir, exist_ok=True)
            kwargs["tmpdir"] = tdir
        except Exception as e:
            print(f"trace setup failed: {e}")

    res = bass_utils.run_bass_kernel_spmd(
        nc, in_maps, core_ids=list(range(NCORES)), **kwargs)
    LAST_EXEC_NS = res.exec_time_ns
    LAST_RES = res

    total = res.results[0]["out"].astype(np.float32).copy()
    for c in range(1, NCORES):
        total += res.results[c]["out"]
    return np.ascontiguousarray(total.T).reshape(B, S, DM)


# revision 39
# speedup vs baseline: 1.2444x; 1.1507x over previous
"""8-core SPMD multi-head attention kernel for Trainium2 (Bass/Tile). v2

Problem: nn.MultiHeadAttention, B=2, S=2048, d_model=1024, 16 heads (dk=64).

Sharding: tensor-parallel over heads - 2 heads per core. Q/K/V projection
weights are column-split per core, out-projection row-split; each core
produces a partial [1024, 4096] output that the host sums.

Design (v1 traced at 408us: ACT did 219us incl. copies, PE stuck at the
HAM cold clock behind phase barriers; this version runs ~273us):
  - ACT (scalar engine) runs only the 128 exp activations [128,1024]
    during attention (scale 1/8 and a shift of -2 folded in; softmax is
    shift-invariant). PSUM->SBUF copies / bias adds run on DVE, plus ACT
    only where exp isn't live. One table set, preloaded by a dummy exp.
  - No phase barriers: one PSUM layout for the whole kernel. pss
    [128,1024]x2bufs (4 banks) + pctx [65,1024] (2) + po [128,512]x2 (2).
    Projections, V-relayout and out-projection all rotate the po tag.
  - Attention: for (b, q-window 1024, head): 16 k-chunks of {2 score MMs
    N=512 (tile_position packs the 64-deep contraction) -> one exp
    [128,1024] -> 2 AV MMs accumulating ctx^T + sums via a ones column}.
    AV emission lags 3 chunks so the first AV's pctx WAR (prev block's
    norm chain) doesn't stall the in-order tensor queue.
  - Batch 1's projections drain as side-work units inside batch 0's
    attention chunk stream; out-projection of each q-window rides in the
    second half of the next window; the final window's out-proj splits
    into two 64-contraction MMs reading the h1 staging tile directly.
  - V is re-laid out token-major via identity matmuls; normalization via
    reciprocal_approx_fast + gpsimd partition_broadcast (sums staged to
    SBUF first: reciprocal_approx_fast misreads PSUM inputs).
fp8 was tried and rejected: quantization error on tensors feeding sums
does not average down (~3.5% rel err > the 2e-2 budget).
"""

import sys

sys.path.insert(0, "/opt/trn_rl_repo")

import numpy as np
import ml_dtypes

import concourse.bass as bass  # noqa: F401
import concourse.mybir as mybir
import concourse.tile as tile
from concourse import bacc
from concourse import bass_utils
from concourse.masks import make_identity

B, S, DM, H, DK = 2, 2048, 1024, 16, 64
TOK = B * S
NCORES = 8
HPC = H // NCORES    # 2 heads per core
CW = HPC * DK        # 128 = per-core qkv width
KC = DM // 128       # 8 contraction chunks
KCH = S // 128       # 16 k-token chunks per batch
F32 = mybir.dt.float32
BF16 = mybir.dt.bfloat16
FP8 = mybir.dt.float8e4
AFT = mybir.ActivationFunctionType
DR = mybir.MatmulPerfMode.DoubleRow
VPAD = 80   # fp8 k-tile stride for vh (16B-aligned; holds DK+1=65 used)

TRACE = False
LAST_EXEC_NS = None
LAST_RES = None

_compiled = None


def _build():
    nc = bacc.Bacc("TRN2", target_bir_lowering=False, debug=False,
                   num_devices=NCORES)

    qT = nc.dram_tensor("qT", [DM, TOK], BF16, kind="ExternalInput").ap()
    kT = nc.dram_tensor("kT", [DM, TOK], BF16, kind="ExternalInput").ap()
    vT = nc.dram_tensor("vT", [DM, TOK], BF16, kind="ExternalInput").ap()
    wq = nc.dram_tensor("wq", [128, KC * CW], BF16, kind="ExternalInput").ap()
    wk = nc.dram_tensor("wk", [128, KC * CW], BF16, kind="ExternalInput").ap()
    wv = nc.dram_tensor("wv", [128, KC * CW], BF16, kind="ExternalInput").ap()
    bq = nc.dram_tensor("bq", [128, 1], F32, kind="ExternalInput").ap()
    bk = nc.dram_tensor("bk", [128, 1], F32, kind="ExternalInput").ap()
    bv = nc.dram_tensor("bv", [128, 1], F32, kind="ExternalInput").ap()
    wo = nc.dram_tensor("wo", [CW, DM], BF16, kind="ExternalInput").ap()
    bo8 = nc.dram_tensor("bo8", [128, 8], F32, kind="ExternalInput").ap()
    out = nc.dram_tensor("out", [DM, TOK], BF16, kind="ExternalOutput").ap()

    xsrc = {"q": qT, "k": kT, "v": vT}

    with tile.TileContext(nc) as tc, \
         tc.tile_pool(name="const", bufs=1) as const, \
         tc.tile_pool(name="xin", bufs=2) as xin, \
         tc.tile_pool(name="expp", bufs=10) as expp, \
         tc.tile_pool(name="stage", bufs=2) as stage, \
         tc.tile_pool(name="outst", bufs=8) as outst, \
         tc.tile_pool(name="ppool", bufs=2, space="PSUM") as ppool, \
         tc.tile_pool(name="pscore", bufs=2, space="PSUM") as pscore, \
         tc.tile_pool(name="pctxp", bufs=1, space="PSUM") as pctxp:

        # ---------- constants & persistent buffers ----------
        # weight DMAs are emitted inside the proj scope so the FIFO DMA
        # ring streams batch-0's K input first (it gates the first MMs)
        w_sb = {}
        b_sb = {}
        wsrc = {"q": wq, "k": wk, "v": wv}
        bsrc = {"q": bq, "k": bk, "v": bv}
        for name in ("q", "k", "v"):
            w_sb[name] = const.tile([128, KC, CW], BF16, tag=f"w{name}",
                                    name=f"w{name}")
            b_sb[name] = const.tile([128, 1], F32, tag=f"b{name}",
                                    name=f"b{name}")

        def weight_dmas(name):
            nc.sync.dma_start(w_sb[name][:],
                              wsrc[name].rearrange("p (c m) -> p c m", c=KC))
            nc.sync.dma_start(b_sb[name][:], bsrc[name][:])

        wo_sb = const.tile([CW, DM], BF16, tag="wo")
        # lower half of Wo re-homed at partitions 0-63: lets the final
        # window's out-proj read the h1 ctx from its staging tile (base 0)
        # via a second 64-contraction accumulate, skipping the ctxT DMA
        wo2_sb = const.tile([DK, DM], BF16, tag="wo2")
        bo_sb = const.tile([128, 8], F32, tag="bo")
        ident = const.tile([128, 128], BF16, tag="ident")
        make_identity(nc, ident[:])

        qhT = [const.tile([128, S], BF16, tag=f"qhT{b}", name=f"qhT{b}")
               for b in range(B)]
        khT = [const.tile([128, S], BF16, tag=f"khT{b}", name=f"khT{b}")
               for b in range(B)]
        vhT = [const.tile([128, S], BF16, tag=f"vhT{b}", name=f"vhT{b}")
               for b in range(B)]
        ctxT = [const.tile([128, S], BF16, tag=f"ctxT{b}", name=f"ctxT{b}")
                for b in range(B)]
        dstT = {"q": qhT, "k": khT, "v": vhT}
        # vh: token-major V with a ones column for the softmax sums
        vh = const.tile([128, B, HPC, KCH, DK + 1], BF16, tag="vh")
        nc.gpsimd.memset(vh[:, :, :, :, DK:DK + 1], 1.0)

        # per-partition -2.0 bias column for the shifted exp
        negtwo = const.tile([128, 1], F32, tag="negtwo")
        nc.gpsimd.memset(negtwo[:], -2.0)

        # dummy exp: trigger the one ACT_TABLE_LOAD during the prologue
        warm = const.tile([1, 1], F32, tag="warm")
        nc.scalar.activation(warm[:], negtwo[0:1, 0:1], AFT.Exp, scale=0.125)

        # ---------- phase 1: projections (k,q then v per batch) ----------
        # po-tag PSUM tiles [128,512] rotate through proj, V-transpose and
        # out-projection uses; alternate DVE/GpSimd for the bias copies.
        copy_flip = 0

        def input_dmas(b, name):
            xt = xin.tile([128, KC, S], BF16, tag="xt", name=f"xt_{name}{b}")
            for c in range(KC):
                nc.sync.dma_start(
                    xt[:, c, :],
                    xsrc[name][c * 128:(c + 1) * 128, b * S:(b + 1) * S])
            return xt

        def proj_tile(b, name, xt, qtr, act_ok):
            nonlocal copy_flip
            pp = ppool.tile([128, 512], F32, tag="po",
                            name=f"pp_{name}{b}_{qtr}")
            for c in range(KC):
                nc.tensor.matmul(
                    pp[:], w_sb[name][:, c, :],
                    xt[:, c, qtr * 512:(qtr + 1) * 512],
                    start=(c == 0), stop=(c == KC - 1))
            d = dstT[name][b][:, qtr * 512:(qtr + 1) * 512]
            if act_ok and copy_flip % 2 == 1:
                nc.scalar.activation(d, pp[:], AFT.Identity,
                                     bias=b_sb[name][:])
            else:
                nc.vector.tensor_scalar_add(d, pp[:], b_sb[name][:])
            copy_flip += 1

        def relayout_tile(b, t, act_ok):
            # vh[tok, dims] = vhT^T via identity matmul, then copy
            nonlocal copy_flip
            pt = ppool.tile([128, 512], F32, tag="po", name=f"pt{b}_{t}")
            nc.tensor.matmul(pt[:, 0:128],
                             vhT[b][:, t * 128:(t + 1) * 128], ident[:])
            src = pt[:, 0:128].rearrange("p (h d) -> p h d", h=HPC)
            dst = vh[:, b, :, t, 0:DK]
            if act_ok and copy_flip % 2 == 1:
                nc.scalar.activation(dst, src, AFT.Copy)
            else:
                nc.vector.tensor_copy(dst, src)
            copy_flip += 1

        # prologue: batch 0 projections serial (ACT is free to help);
        # batch 1 projections become side-work units drained one per
        # chunk inside batch 0's attention stream.
        with nc.named_scope("proj"):
            xts = {}
            weight_dmas("k")
            xts["k"] = input_dmas(0, "k")
            weight_dmas("v")
            weight_dmas("q")
            for nm in ("v", "q"):
                xts[nm] = input_dmas(0, nm)
            # out-proj weights: first needed ~90us in, after b0's inputs
            nc.sync.dma_start(wo_sb[:], wo[:])
            nc.sync.dma_start(wo2_sb[:], wo[DK:2 * DK, :])
            nc.sync.dma_start(bo_sb[:], bo8[:])
            for qtr in range(4):
                proj_tile(0, "k", xts["k"], qtr, act_ok=True)
            for qtr in range(4):
                proj_tile(0, "v", xts["v"], qtr, act_ok=True)
            for t in range(KCH):
                relayout_tile(0, t, act_ok=True)
            for qtr in range(4):
                proj_tile(0, "q", xts["q"], qtr, act_ok=True)
            for nm in ("k", "v", "q"):
                xts[nm] = input_dmas(1, nm)

        side_work = []
        for qtr in range(4):
            side_work.append(lambda qtr=qtr: proj_tile(
                1, "k", xts["k"], qtr, act_ok=False))
        for vt in range(4):
            side_work.append(lambda vt=vt: proj_tile(
                1, "v", xts["v"], vt, act_ok=False))
            for t in range(vt * 4, vt * 4 + 4):
                side_work.append(lambda t=t: relayout_tile(
                    1, t, act_ok=False))
        for qtr in range(4):
            side_work.append(lambda qtr=qtr: proj_tile(
                1, "q", xts["q"], qtr, act_ok=False))
        side_work.reverse()   # pop() from the front-most unit

        # ---------- phase 2: attention with interleaved out-proj ----------
        # blocks: (b, qw, h); 16 chunks each. Out-proj of the previous
        # q-window (8 od-matmuls) rides along every 4th chunk.
        oproj_queue = []   # (b, qw) blocks whose ctxT is complete
        cs_tiles = {}      # (b, qw) -> h1 ctx staging tile

        def emit_oproj(n, act_ok=False):
            # emit n pending out-projection column-chunks, if any
            for i in range(n):
                if not oproj_queue:
                    return
                ob, oqw, od = oproj_queue.pop(0)
                col = od
                po = ppool.tile([128, 512], F32, tag="po",
                                name=f"po{ob}_{oqw}_{od}")
                q0 = oqw * 512
                if act_ok and (ob, oqw) in cs_tiles:
                    cs = cs_tiles[(ob, oqw)]
                    nc.tensor.matmul(
                        po[:], wo_sb[0:DK, col * 128:(col + 1) * 128],
                        ctxT[ob][0:DK, q0:q0 + 512],
                        start=True, stop=False)
                    nc.tensor.matmul(
                        po[:], wo2_sb[:, col * 128:(col + 1) * 128],
                        cs[:],
                        start=False, stop=True)
                else:
                    nc.tensor.matmul(
                        po[:], wo_sb[:, col * 128:(col + 1) * 128],
                        ctxT[ob][:, q0:q0 + 512])
                ot = outst.tile([128, 512], BF16, tag="ot",
                                name=f"ot{ob}_{oqw}_{od}")
                if act_ok and i % 2 == 1:
                    nc.scalar.activation(ot[:], po[:], AFT.Identity,
                                         bias=bo_sb[:, col:col + 1])
                else:
                    nc.vector.tensor_scalar_add(
                        ot[:], po[:], bo_sb[:, col:col + 1])
                nc.sync.dma_start(
                    out[col * 128:(col + 1) * 128,
                        ob * S + q0:ob * S + q0 + 512], ot[:])

        with nc.named_scope("attn"):
            for b in range(B):
                for qw in range(4):          # 512-wide q windows
                    q0 = qw * 512
                    pctx = [pctxp.tile([DK + 1, 512], F32, tag=f"pctx{h}",
                                       name=f"pctx{b}_{qw}_{h}")
                            for h in range(HPC)]
                    etiles = {}

                    def do_av(c):
                        ec = etiles.pop(c)
                        for h in range(HPC):
                            nc.tensor.matmul(
                                pctx[h][:], vh[:, b, h, c, :],
                                ec[:, h * 512:(h + 1) * 512],
                                start=(c == 0), stop=(c == KCH - 1))

                    # AV emission lags the score/exp stream by AVLAG
                    # chunks so the first AV (which WAR-waits the prev
                    # window's pctx through its norm chain) enters the
                    # in-order tensor queue after that chain has drained.
                    AVLAG = 3
                    for c in range(KCH):
                        pss = pscore.tile([128, 1024], F32, tag="pss",
                                          name=f"pss{b}_{qw}_{c}")
                        # both heads per chunk: alternating tile_position
                        # row groups let the packed 64-deep score matmuls
                        # overlap inside the PE array
                        for h in range(HPC):
                            hb = h * DK
                            nc.tensor.matmul(
                                pss[:, h * 512:(h + 1) * 512],
                                khT[b][hb:hb + DK, c * 128:(c + 1) * 128],
                                qhT[b][hb:hb + DK, q0:q0 + 512],
                                tile_position=(hb, 0))
                        e = expp.tile([128, 1024], BF16, tag="e",
                                      name=f"e{b}_{qw}_{c}")
                        # bias -2: softmax is shift-invariant (the sums
                        # row scales identically); keeps exp small
                        nc.scalar.activation(e[:], pss[:], AFT.Exp,
                                             bias=negtwo[:], scale=0.125)
                        etiles[c] = e
                        if c >= AVLAG:
                            do_av(c - AVLAG)
                        # batch-1 projection side-work keeps the PE fed
                        # through batch 0's attention; slots track the
                        # input-DMA arrival order (in-order PE queue).
                        gchunk = qw * KCH + c
                        if b == 0 and 8 <= gchunk and side_work:
                            side_work.pop()()
                        # out-proj of the previous q-window rides in the
                        # second half of this window so its ctxT (incl.
                        # the norm chain + h1 DMA) is ready — an early
                        # emit stalls the in-order tensor queue.
                        if 8 <= c:
                            emit_oproj(1)
                    for c in range(KCH - AVLAG, KCH):
                        do_av(c)
                    # normalization: ctx / sums (row DK of pctx).
                    # NB: reciprocal_approx_fast reading PSUM directly
                    # gives garbage — stage the sums row in SBUF first.
                    for h in range(HPC):
                        ssum = stage.tile([1, 512], F32, tag=f"ssum{h}",
                                          name=f"ssum{b}_{qw}_{h}")
                        nc.vector.tensor_copy(ssum[:], pctx[h][DK:DK + 1, :])
                        si = stage.tile([1, 512], F32, tag=f"si{h}",
                                        name=f"si{b}_{qw}_{h}")
                        nc.vector.reciprocal_approx_fast(si[:], ssum[:])
                        sbc = stage.tile([DK, 512], F32, tag=f"sbc{h}",
                                         name=f"sbc{b}_{qw}_{h}")
                        nc.gpsimd.partition_broadcast(sbc[:], si[:])
                        if h == 0:
                            nc.vector.tensor_mul(
                                ctxT[b][0:DK, q0:q0 + 512],
                                pctx[0][0:DK, :], sbc[:])
                        else:
                            cs = stage.tile([DK, 512], BF16, tag="cs",
                                            name=f"cs{b}_{qw}")
                            nc.vector.tensor_mul(cs[:], pctx[1][0:DK, :],
                                                 sbc[:])
                            if (b, qw) == (B - 1, 3):
                                # final window: tail out-proj reads cs
                                # directly; no DMA needed
                                cs_tiles[(b, qw)] = cs
                            else:
                                # gpsimd SWDGE ring: keeps this off the
                                # sync ring so the next window's out-proj
                                # isn't stuck behind queued output DMAs
                                nc.gpsimd.dma_start(
                                    ctxT[b][DK:2 * DK, q0:q0 + 512], cs[:])
                    for od in range(8):
                        oproj_queue.append((b, qw, od))

        with nc.named_scope("tail"):
            emit_oproj(len(oproj_queue), act_ok=True)

    nc.compile()
    return nc


def _get_compiled():
    global _compiled
    if _compiled is None:
        _compiled = _build()
    return _compiled


def _xT(x):
    xf = np.asarray(x, np.float32).reshape(TOK, DM)
    return np.ascontiguousarray(xf.T).astype(ml_dtypes.bfloat16)


def _wshuf(W, cs):
    # [1024, 128] core slice -> [p, c*128+m] so SBUF [128, KC, CW] DMAs clean
    Wc = np.asarray(W, np.float32)[:, cs]
    return np.ascontiguousarray(
        Wc.reshape(KC, 128, CW).transpose(1, 0, 2).reshape(128, KC * CW)
    ).astype(ml_dtypes.bfloat16)


def kernel(q, k, v, Wq, bq, Wk, bk, Wv, bv, Wo, bo):
    global LAST_EXEC_NS, LAST_RES
    nc = _get_compiled()

    qTa, kTa, vTa = _xT(q), _xT(k), _xT(v)

    bq, bk, bv = (np.asarray(a, np.float32) for a in (bq, bk, bv))
    Wo = np.asarray(Wo, np.float32)
    bo = np.asarray(bo, np.float32)

    in_maps = []
    for c in range(NCORES):
        cs = slice(c * CW, (c + 1) * CW)
        in_maps.append({
            "qT": qTa, "kT": kTa, "vT": vTa,
            "wq": _wshuf(Wq, cs), "wk": _wshuf(Wk, cs), "wv": _wshuf(Wv, cs),
            "bq": np.ascontiguousarray(bq[cs].reshape(CW, 1)),
            "bk": np.ascontiguousarray(bk[cs].reshape(CW, 1)),
            "bv": np.ascontiguousarray(bv[cs].reshape(CW, 1)),
            "wo": np.ascontiguousarray(Wo[cs, :]).astype(ml_dtypes.bfloat16),
            "bo8": np.ascontiguousarray((bo / NCORES).reshape(8, 128).T),
        })

    kwargs = {}
    if TRACE:
        try:
            import os
            import shutil
            import ntff_shim
            ntff_shim.install()
            kwargs["trace"] = True
            tdir = "/tmp/bass_trace"
            shutil.rmtree(tdir, ignore_errors=True)
            os.makedirs(tdir, exist_ok=True)
            kwargs["tmpdir"] = tdir
        except Exception as e:
            print(f"trace setup failed: {e}")

    res = bass_utils.run_bass_kernel_spmd(
        nc, in_maps, core_ids=list(range(NCORES)), **kwargs)
    LAST_EXEC_NS = res.exec_time_ns
    LAST_RES = res

    total = res.results[0]["out"].astype(np.float32).copy()
    for c in range(1, NCORES):
        total += res.results[c]["out"]
    return np.ascontiguousarray(total.T).reshape(B, S, DM)
